# revision 21
# baseline (speedup 1.0000x reference)
"""HETXLHead forward on 8 Trainium2 NeuronCores (Bass/Tile).

Reference computation (per batch row b, S=64 samples, D=512, R=16, C=1000):
    lrc   = (f @ W_cov.T + b_cov).reshape(B, D, R)
    ds    = softplus(f @ W_diag.T + b_diag) + 1e-3
    smp   = einsum('bdr,bsr->bsd', lrc, nlr) + ds[:,None,:] * nd
    out   = ((f[:,None,:] + smp) @ W_cls.T + b_cls) / 1.5

Sharding: data-parallel over B=1024 -> 128 rows per core; weights replicated.

Device pipeline per core (tokens t = 64*b + s, 8192 per core; all heavy
streams host-cast to bf16 to halve HBM traffic):
  A) cov[b, j'] = f @ W_cov'.T via PE, j' = 512*r + d (r-major, host-permuted)
  B) dsT[d, b] = softplus(W_diag @ f.T + b_diag) + 1e-3 (ACT Softplus direct)
  C) one SBUF->SBUF shuffle DMA per b-group of 8 turns cov rows into the
     block-diag lhsT ls[(g,r), d]; PSUM X = ls.T @ bdt (host-built block-diag
     noise_lr) + f via a K=8 ones-block matmul; noise_diag arrives HOST-
     TRANSPOSED [d, t] so the diag term is one broadcast DVE multiply per
     [128,512] tile (dsT column broadcast over each b's 64 sample columns);
     DVE adds PSUM X + diag -> pret (bf16).
  D) logits[t, c] = pret.T @ (W_cls.T/1.5) in bf16, ACT-copy PSUM->SBUF f32,
     one batched DMA out per b-group.
"""

import os
import sys

for _p in ("/opt/trn_rl_repo", "/opt/pypackages"):
    if os.path.isdir(_p) and _p not in sys.path:
        sys.path.insert(0, _p)

import numpy as np

import concourse.bass as bass
import concourse.bacc as bacc
import concourse.mybir as mybir
import concourse.tile as tile
from concourse.bass import broadcast_tensor_aps
from concourse.bass_utils import run_bass_kernel_spmd

F32 = mybir.dt.float32
BF16 = mybir.dt.bfloat16
NP_BF16 = mybir.dt.np(BF16)
AF = mybir.ActivationFunctionType

N_CORES = 8
B_FULL, D, R, S, C = 1024, 512, 16, 64, 1000
NB = B_FULL // N_CORES        # 128 batch rows per core
NT = NB * S                   # 8192 tokens per core
NDC = D // 128                # 4 d-chunks
NEC = D // 128                # 4 e-chunks (contraction of the input feature dim)
NBG = NB // 8                 # 16 b-groups of 8
NJC = R                       # 16 j'-chunks of 512 (one per r)
MIN_SCALE = 1e-3


def _build(has_bcov: bool, has_bcls: bool):
    nc = bacc.Bacc(None, target_bir_lowering=False, debug=True)

    d_ft = nc.dram_tensor("f_t", [D, NB], BF16, kind="ExternalInput")
    d_wcov = nc.dram_tensor("w_cov_t", [D, D * R], BF16, kind="ExternalInput")
    d_wdiag = nc.dram_tensor("w_diag_t", [D, D], BF16, kind="ExternalInput")
    d_wcls = nc.dram_tensor("w_cls_t", [D, C], BF16, kind="ExternalInput")
    d_bdiag = nc.dram_tensor("b_diag_col", [128, NDC], F32, kind="ExternalInput")
    d_bdt = nc.dram_tensor("bdt", [128, NBG * 512], BF16, kind="ExternalInput")
    d_noise = nc.dram_tensor("noise_t", [D, NT], BF16, kind="ExternalInput")
    if has_bcov:
        d_bcov = nc.dram_tensor("bcov_tiled", [128, D], BF16, kind="ExternalInput")
    if has_bcls:
        d_bcls = nc.dram_tensor("bcls_bcast", [128, C], F32, kind="ExternalInput")
    d_out = nc.dram_tensor("out", [NT, C], BF16, kind="ExternalOutput")

    with tile.TileContext(nc) as tc:
        with (
            tc.tile_pool(name="const", bufs=1) as const_pool,
            tc.tile_pool(name="wstream", bufs=3) as wstream_pool,
            tc.tile_pool(name="ls", bufs=6) as ls_pool,
            tc.tile_pool(name="noise", bufs=4) as noise_pool,
            tc.tile_pool(name="tmp", bufs=6) as tmp_pool,
            tc.tile_pool(name="pret", bufs=8) as pret_pool,
            tc.tile_pool(name="lout", bufs=6) as lout_pool,
            tc.tile_pool(name="ps_x", bufs=3, space="PSUM") as ps_x,
            tc.tile_pool(name="ps_l", bufs=2, space="PSUM") as ps_l,
        ):
            # ---- constants into SBUF (batched loads) ----
            # W_cov-dependent work gates the whole pipeline, so its stream
            # (scalar ring) starts first; bulky consts not needed until the
            # bg loop (wcls, bdt) are emitted after the cov loop below.
            ft_sb = const_pool.tile([128, NEC * NB], BF16, tag="ft")
            nc.sync.dma_start(
                ft_sb[:], d_ft.rearrange("(ec p) b -> p ec b", ec=NEC)
            )
            wd_sb = const_pool.tile([128, NEC * D], BF16, tag="wd")
            nc.sync.dma_start(
                wd_sb[:], d_wdiag.rearrange("(ec p) d -> p ec d", ec=NEC)
            )
            bdiag = const_pool.tile([128, NDC], F32, tag="bdiag")
            nc.sync.dma_start(bdiag[:], d_bdiag[:, :])
            # bg-loop consts next on the sync ring: needed by ~t=28us, while
            # the wcov stream owns the scalar ring
            bdt_sb = const_pool.tile([128, NBG * 512], BF16, tag="bdt")
            nc.sync.dma_start(bdt_sb[:], d_bdt[:, :])
            wcls_sb = const_pool.tile([128, NDC * C], BF16, tag="wcls")
            nc.sync.dma_start(
                wcls_sb[:], d_wcls.rearrange("(dc p) c -> p dc c", dc=NDC)
            )
            if has_bcov:
                bcov = const_pool.tile([128, D], BF16, tag="bcov")
                nc.sync.dma_start(bcov[:], d_bcov[:, :])
            if has_bcls:
                bcls = const_pool.tile([128, C], F32, tag="bcls")
                nc.sync.dma_start(bcls[:], d_bcls[:, :])

            def ft(ec):
                return ft_sb[:, ec * NB:(ec + 1) * NB]

            # ---- B) dsT[d, b] = softplus(W_diag @ f.T + b_diag) + 1e-3 ----
            dst = []
            for dc in range(NDC):
                pd_full = ps_x.tile([128, 512], F32, tag="px")
                pd = pd_full[:, 0:NB]
                for ec in range(NEC):
                    nc.tensor.matmul(
                        pd[:],
                        wd_sb[:, ec * D + dc * 128:ec * D + (dc + 1) * 128],
                        ft(ec),
                        start=(ec == 0),
                        stop=(ec == NEC - 1),
                    )
                t = const_pool.tile([128, NB], F32, tag=f"dst{dc}")
                # softplus(x) = Ln(1 + Exp(x)); x = pd + b_diag column
                nc.scalar.activation(
                    t[:], pd[:], AF.Exp, bias=bdiag[:, dc:dc + 1], scale=1.0
                )
                nc.vector.tensor_scalar_add(t[:], t[:], 1.0)
                nc.scalar.activation(t[:], t[:], AF.Ln)
                nc.vector.tensor_scalar_add(t[:], t[:], MIN_SCALE)
                dst.append(t)

            # ---- A) cov[b, j'] (j' = 512r + d, r-major) ----
            cov = const_pool.tile([NB, D * R], BF16, tag="cov")
            wcov_r = d_wcov.rearrange("(ec p) j -> p ec j", ec=NEC)
            for jc in range(NJC):
                wt = wstream_pool.tile([128, NEC * 512], BF16)
                nc.scalar.dma_start(
                    wt[:], wcov_r[:, :, jc * 512:(jc + 1) * 512]
                )
                pa = ps_x.tile([128, 512], F32, tag="px")
                for ec in range(NEC):
                    nc.tensor.matmul(
                        pa[:], ft(ec), wt[:, ec * 512:(ec + 1) * 512],
                        start=(ec == 0), stop=(ec == NEC - 1),
                    )
                nc.vector.tensor_copy(cov[:, jc * 512:(jc + 1) * 512], pa[:])

            # ---- C + D pipeline over b-groups ----
            noise_r = d_noise.rearrange("(dc p) t -> p dc t", dc=NDC)
            for bg in range(NBG):
                # block-diag lhsT ls[(g,r), d] via one SBUF->SBUF shuffle DMA
                ls = ls_pool.tile([128, D], BF16)
                nc.gpsimd.dma_start(
                    ls[:],
                    cov[bg * 8:(bg + 1) * 8, :].rearrange("p (r d) -> p r d", r=R),
                )
                # noise tiles for this b-group's 512 tokens, [d, t] layout
                # (scalar ring: queues behind wcov, ahead of nothing urgent)
                nz = noise_pool.tile([128, NDC * 512], BF16)
                nc.scalar.dma_start(
                    nz[:], noise_r[:, :, bg * 512:(bg + 1) * 512]
                )

                bdt_bg = bdt_sb[:, bg * 512:(bg + 1) * 512]
                pret = []
                for dc in range(NDC):
                    px = ps_x.tile([128, 512], F32, tag="px")
                    nc.tensor.matmul(
                        px[:],
                        ls[:, dc * 128:(dc + 1) * 128],
                        bdt_bg,
                        start=True,
                        stop=not has_bcov,
                    )
                    if has_bcov:
                        nc.tensor.matmul(
                            px[:],
                            bcov[:, dc * 128:(dc + 1) * 128],
                            bdt_bg,
                            start=False,
                            stop=True,
                        )

                    # diag term ds*noise: one broadcast multiply per
                    # [128,512] tile on the otherwise-idle GpSimd engine,
                    # then + f (broadcast per b) alternating DVE/GpSimd
                    tmp = tmp_pool.tile([128, 512], BF16)
                    n3 = nz[:, dc * 512:(dc + 1) * 512].rearrange(
                        "p (b s) -> p b s", b=8
                    )
                    ds3 = dst[dc][:, bg * 8:(bg + 1) * 8].rearrange(
                        "p (b o) -> p b o", o=1
                    )
                    n3b, ds3b = broadcast_tensor_aps(n3, ds3)
                    t3 = tmp[:].rearrange("p (b s) -> p b s", b=8)
                    nc.gpsimd.tensor_mul(t3, n3b, ds3b)

                    f3 = ft_sb[
                        :, dc * NB + bg * 8:dc * NB + (bg + 1) * 8
                    ].rearrange("p (b o) -> p b o", o=1)
                    _, f3b = broadcast_tensor_aps(n3, f3)
                    eng = nc.vector if dc % 2 == 0 else nc.gpsimd
                    eng.tensor_add(t3, t3, f3b)

                    pt = pret_pool.tile([128, 512], BF16)
                    nc.vector.tensor_add(pt[:], px[:], tmp[:])
                    pret.append(pt)

                # D) logits for the 4 token-tiles of this b-group.
                # Two single-bank PSUM halves per ti; the PSUM->SBUF copies
                # split across ACT and DVE so neither gates the PE.
                for ti in range(4):
                    lo = lout_pool.tile([128, C], BF16)
                    pa_l = ps_l.tile([128, 512], F32)
                    pb_l = ps_l.tile([128, 512], F32)
                    pb = pb_l[:, 0:C - 512]
                    for dc in range(NDC):
                        stat = pret[dc][:, ti * 128:(ti + 1) * 128]
                        nc.tensor.matmul(
                            pa_l[:], stat, wcls_sb[:, dc * C:dc * C + 512],
                            start=(dc == 0), stop=(dc == NDC - 1),
                        )
                        nc.tensor.matmul(
                            pb[:],
                            stat,
                            wcls_sb[:, dc * C + 512:(dc + 1) * C],
                            start=(dc == 0), stop=(dc == NDC - 1),
                        )
                    if has_bcls:
                        nc.vector.tensor_add(lo[:, 0:512], pa_l[:], bcls[:, 0:512])
                        nc.vector.tensor_add(lo[:, 512:C], pb[:], bcls[:, 512:C])
                    else:
                        nc.scalar.activation(lo[:, 0:512], pa_l[:], AF.Copy)
                        nc.vector.tensor_copy(lo[:, 512:C], pb[:])
                    nc.sync.dma_start(
                        d_out[bg * 512 + ti * 128:bg * 512 + (ti + 1) * 128, :],
                        lo[:],
                    )

    nc.compile()
    return nc


def build_in_maps(
    features, W_cov, b_cov, W_diag, b_diag, W_cls, b_cls, noise_diag, noise_lr
):
    features = np.asarray(features, np.float32)
    W_cov = np.asarray(W_cov, np.float32)
    b_cov = np.asarray(b_cov, np.float32)
    W_diag = np.asarray(W_diag, np.float32)
    b_diag = np.asarray(b_diag, np.float32)
    W_cls = np.asarray(W_cls, np.float32)
    b_cls = np.asarray(b_cls, np.float32)
    noise_diag = np.asarray(noise_diag, np.float32)
    noise_lr = np.asarray(noise_lr, np.float32)

    has_bcov = bool(np.any(b_cov))
    has_bcls = bool(np.any(b_cls))

    # host-side layout prep (replicated weights)
    # W_cov[j=16d+r, e] -> [e, j'=512r+d]
    w_cov_t = np.ascontiguousarray(
        W_cov.reshape(D, R, D).transpose(2, 1, 0).reshape(D, D * R)
    ).astype(NP_BF16)
    w_diag_t = np.ascontiguousarray(W_diag.T).astype(NP_BF16)
    w_cls_t = np.ascontiguousarray(W_cls.T / 1.5).astype(NP_BF16)
    b_diag_col = np.ascontiguousarray(b_diag.reshape(NDC, 128).T)

    common = {
        "w_cov_t": w_cov_t,
        "w_diag_t": w_diag_t,
        "w_cls_t": w_cls_t,
        "b_diag_col": b_diag_col,
    }
    if has_bcov:
        # b_cov[j=16d+r] -> [r, d] tiled 8x along partitions -> [128, 512]
        bcov_r = b_cov.reshape(D, R).T
        common["bcov_tiled"] = np.ascontiguousarray(np.tile(bcov_r, (8, 1))).astype(
            NP_BF16
        )
    if has_bcls:
        common["bcls_bcast"] = np.ascontiguousarray(
            np.tile((b_cls / 1.5)[None, :], (128, 1))
        )

    in_maps = []
    for c in range(N_CORES):
        b0 = c * NB
        sl = slice(b0, b0 + NB)
        m = dict(common)
        m["f_t"] = np.ascontiguousarray(features[sl].T).astype(NP_BF16)
        # block-diag noise_lr: per bg a [128=(g,r), 512=(g,s)] tile
        nlr_c = noise_lr[sl].reshape(NBG, 8, S, R)
        bdt = np.zeros((NBG, 8, R, 8, S), np.float32)
        for g in range(8):
            bdt[:, g, :, g, :] = nlr_c[:, g].transpose(0, 2, 1)
        m["bdt"] = np.ascontiguousarray(
            bdt.reshape(NBG, 128, 512).transpose(1, 0, 2).reshape(128, NBG * 512)
        ).astype(NP_BF16)
        # noise transposed to [d, t]
        m["noise_t"] = np.ascontiguousarray(
            noise_diag[sl].reshape(NT, D).T
        ).astype(NP_BF16)
        in_maps.append(m)

    return in_maps, has_bcov, has_bcls


def kernel(**inputs):
    in_maps, has_bcov, has_bcls = build_in_maps(**inputs)
    nc = _build(has_bcov, has_bcls)
    res = run_bass_kernel_spmd(nc, in_maps, list(range(N_CORES)))
    out = np.concatenate(
        [np.asarray(res.results[c]["out"]).astype(np.float32) for c in range(N_CORES)],
        axis=0,
    )
    return out.reshape(B_FULL, S, C)


if __name__ == "__main__":
    rng = np.random.default_rng(0)
    ins = {
        "features": rng.standard_normal((B_FULL, D), np.float32),
        "W_cov": rng.standard_normal((D * R, D), np.float32) / np.sqrt(D),
        "b_cov": np.zeros((D * R,), np.float32),
        "W_diag": rng.standard_normal((D, D), np.float32) / np.sqrt(D),
        "b_diag": np.zeros((D,), np.float32),
        "W_cls": rng.standard_normal((C, D), np.float32) / np.sqrt(D),
        "b_cls": np.zeros((C,), np.float32),
        "noise_diag": rng.standard_normal((B_FULL, S, D), np.float32),
        "noise_lr": rng.standard_normal((B_FULL, S, R), np.float32),
    }
    out = kernel(**ins)
    print(out.shape, out.dtype)


# revision 26
# speedup vs baseline: 1.1339x; 1.1339x over previous
"""HETXLHead forward on 8 Trainium2 NeuronCores (Bass/Tile).

Reference computation (per batch row b, S=64 samples, D=512, R=16, C=1000):
    lrc   = (f @ W_cov.T + b_cov).reshape(B, D, R)
    ds    = softplus(f @ W_diag.T + b_diag) + 1e-3
    smp   = einsum('bdr,bsr->bsd', lrc, nlr) + ds[:,None,:] * nd
    out   = ((f[:,None,:] + smp) @ W_cls.T + b_cls) / 1.5

Sharding: data-parallel over B=1024 -> 128 rows per core; weights replicated.

Device pipeline per core (tokens t = 64*b + s, 8192 per core; all heavy
streams host-cast to bf16 to halve HBM traffic):
  A) cov[b, j'] = f @ W_cov'.T via PE, j' = 512*r + d (r-major, host-permuted)
  B) dsT[d, b] = softplus(W_diag @ f.T + b_diag) + 1e-3 (ACT Softplus direct)
  C) one SBUF->SBUF shuffle DMA per b-group of 8 turns cov rows into the
     block-diag lhsT ls[(g,r), d]; PSUM X = ls.T @ bdt (host-built block-diag
     noise_lr) + f via a K=8 ones-block matmul; noise_diag arrives HOST-
     TRANSPOSED [d, t] so the diag term is one broadcast DVE multiply per
     [128,512] tile (dsT column broadcast over each b's 64 sample columns);
     DVE adds PSUM X + diag -> pret (bf16).
  D) logits[t, c] = pret.T @ (W_cls.T/1.5) in bf16, ACT-copy PSUM->SBUF f32,
     one batched DMA out per b-group.
"""

import os
import sys

for _p in ("/opt/trn_rl_repo", "/opt/pypackages"):
    if os.path.isdir(_p) and _p not in sys.path:
        sys.path.insert(0, _p)

import numpy as np

import concourse.bass as bass
import concourse.bacc as bacc
import concourse.mybir as mybir
import concourse.tile as tile
from concourse.bass import broadcast_tensor_aps
from concourse.bass_utils import run_bass_kernel_spmd

F32 = mybir.dt.float32
BF16 = mybir.dt.bfloat16
NP_BF16 = mybir.dt.np(BF16)
AF = mybir.ActivationFunctionType

N_CORES = 8
B_FULL, D, R, S, C = 1024, 512, 16, 64, 1000
NB = B_FULL // N_CORES        # 128 batch rows per core
NT = NB * S                   # 8192 tokens per core
NDC = D // 128                # 4 d-chunks
NEC = D // 128                # 4 e-chunks (contraction of the input feature dim)
NBG = NB // 8                 # 16 b-groups of 8
NJC = R                       # 16 j'-chunks of 512 (one per r)
MIN_SCALE = 1e-3


def _build(has_bcov: bool, has_bcls: bool):
    nc = bacc.Bacc(None, target_bir_lowering=False, debug=True)

    d_ft = nc.dram_tensor("f_t", [D, NB], BF16, kind="ExternalInput")
    d_fgs = nc.dram_tensor("f_gs", [8, NBG * D], BF16, kind="ExternalInput")
    d_eye8 = nc.dram_tensor("eye8", [8, 512], BF16, kind="ExternalInput")
    d_wcov = nc.dram_tensor("w_cov_t", [D, D * R], BF16, kind="ExternalInput")
    d_wdiag = nc.dram_tensor("w_diag_t", [D, D], BF16, kind="ExternalInput")
    d_wcls = nc.dram_tensor("w_cls_t", [D, C], BF16, kind="ExternalInput")
    d_bdiag = nc.dram_tensor("b_diag_col", [128, NDC], F32, kind="ExternalInput")
    d_bdt = nc.dram_tensor("bdt", [128, NBG * 512], BF16, kind="ExternalInput")
    d_noise = nc.dram_tensor("noise_t", [D, NT], BF16, kind="ExternalInput")
    if has_bcov:
        d_bcov = nc.dram_tensor("bcov_tiled", [128, D], BF16, kind="ExternalInput")
    if has_bcls:
        d_bcls = nc.dram_tensor("bcls_bcast", [128, C], F32, kind="ExternalInput")
    d_out = nc.dram_tensor("out", [NT, C], BF16, kind="ExternalOutput")

    with tile.TileContext(nc) as tc:
        with (
            tc.tile_pool(name="const", bufs=1) as const_pool,
            tc.tile_pool(name="wstream", bufs=3) as wstream_pool,
            tc.tile_pool(name="ls", bufs=6) as ls_pool,
            tc.tile_pool(name="noise", bufs=4) as noise_pool,
            tc.tile_pool(name="tmp", bufs=6) as tmp_pool,
            tc.tile_pool(name="pret", bufs=8) as pret_pool,
            tc.tile_pool(name="lout", bufs=6) as lout_pool,
            tc.tile_pool(name="ps_x", bufs=3, space="PSUM") as ps_x,
            tc.tile_pool(name="ps_l", bufs=2, space="PSUM") as ps_l,
        ):
            # ---- constants into SBUF (batched loads) ----
            # W_cov-dependent work gates the whole pipeline, so its stream
            # (scalar ring) starts first; bulky consts not needed until the
            # bg loop (wcls, bdt) are emitted after the cov loop below.
            ft_sb = const_pool.tile([128, NEC * NB], BF16, tag="ft")
            nc.sync.dma_start(
                ft_sb[:], d_ft.rearrange("(ec p) b -> p ec b", ec=NEC)
            )
            wd_sb = const_pool.tile([128, NEC * D], BF16, tag="wd")
            nc.sync.dma_start(
                wd_sb[:], d_wdiag.rearrange("(ec p) d -> p ec d", ec=NEC)
            )
            bdiag = const_pool.tile([128, NDC], F32, tag="bdiag")
            nc.sync.dma_start(bdiag[:], d_bdiag[:, :])
            fgs = const_pool.tile([8, NBG * D], BF16, tag="fgs")
            nc.sync.dma_start(fgs[:], d_fgs[:, :])
            eye8 = const_pool.tile([8, 512], BF16, tag="eye8")
            nc.sync.dma_start(eye8[:], d_eye8[:, :])
            # bg-loop consts next on the sync ring: needed by ~t=28us, while
            # the wcov stream owns the scalar ring
            bdt_sb = const_pool.tile([128, NBG * 512], BF16, tag="bdt")
            nc.sync.dma_start(bdt_sb[:], d_bdt[:, :])
            wcls_sb = const_pool.tile([128, NDC * C], BF16, tag="wcls")
            nc.sync.dma_start(
                wcls_sb[:], d_wcls.rearrange("(dc p) c -> p dc c", dc=NDC)
            )
            if has_bcov:
                bcov = const_pool.tile([128, D], BF16, tag="bcov")
                nc.sync.dma_start(bcov[:], d_bcov[:, :])
            if has_bcls:
                bcls = const_pool.tile([128, C], F32, tag="bcls")
                nc.sync.dma_start(bcls[:], d_bcls[:, :])

            def ft(ec):
                return ft_sb[:, ec * NB:(ec + 1) * NB]

            # ---- B) dsT[d, b] = softplus(W_diag @ f.T + b_diag) + 1e-3 ----
            dst = []
            for dc in range(NDC):
                pd_full = ps_x.tile([128, 512], F32, tag="px")
                pd = pd_full[:, 0:NB]
                for ec in range(NEC):
                    nc.tensor.matmul(
                        pd[:],
                        wd_sb[:, ec * D + dc * 128:ec * D + (dc + 1) * 128],
                        ft(ec),
                        start=(ec == 0),
                        stop=(ec == NEC - 1),
                    )
                t = const_pool.tile([128, NB], F32, tag=f"dst{dc}")
                # softplus(x) = Ln(1 + Exp(x)); x = pd + b_diag column
                nc.scalar.activation(
                    t[:], pd[:], AF.Exp, bias=bdiag[:, dc:dc + 1], scale=1.0
                )
                nc.vector.tensor_scalar_add(t[:], t[:], 1.0)
                nc.scalar.activation(t[:], t[:], AF.Ln)
                nc.vector.tensor_scalar_add(t[:], t[:], MIN_SCALE)
                dst.append(t)

            # ---- A) cov[b, j'] (j' = 512r + d, r-major) ----
            cov = const_pool.tile([NB, D * R], BF16, tag="cov")
            wcov_r = d_wcov.rearrange("(ec p) j -> p ec j", ec=NEC)
            for jc in range(NJC):
                wt = wstream_pool.tile([128, NEC * 512], BF16)
                nc.scalar.dma_start(
                    wt[:], wcov_r[:, :, jc * 512:(jc + 1) * 512]
                )
                pa = ps_x.tile([128, 512], F32, tag="px")
                for ec in range(NEC):
                    nc.tensor.matmul(
                        pa[:], ft(ec), wt[:, ec * 512:(ec + 1) * 512],
                        start=(ec == 0), stop=(ec == NEC - 1),
                    )
                nc.vector.tensor_copy(cov[:, jc * 512:(jc + 1) * 512], pa[:])

            # ---- C + D pipeline over b-groups ----
            noise_r = d_noise.rearrange("(dc p) t -> p dc t", dc=NDC)
            for bg in range(NBG):
                # block-diag lhsT ls[(g,r), d] via one SBUF->SBUF shuffle DMA
                ls = ls_pool.tile([128, D], BF16)
                nc.gpsimd.dma_start(
                    ls[:],
                    cov[bg * 8:(bg + 1) * 8, :].rearrange("p (r d) -> p r d", r=R),
                )
                # noise tiles for this b-group's 512 tokens, [d, t] layout
                # (scalar ring: queues behind wcov, ahead of nothing urgent)
                nz = noise_pool.tile([128, NDC * 512], BF16)
                nc.scalar.dma_start(
                    nz[:], noise_r[:, :, bg * 512:(bg + 1) * 512]
                )

                bdt_bg = bdt_sb[:, bg * 512:(bg + 1) * 512]
                pret = []
                for dc in range(NDC):
                    px = ps_x.tile([128, 512], F32, tag="px")
                    n_acc = 3 if has_bcov else 2
                    nc.tensor.matmul(
                        px[:],
                        fgs[:, bg * D + dc * 128:bg * D + (dc + 1) * 128],
                        eye8[:],
                        start=True,
                        stop=(n_acc == 1),
                    )
                    nc.tensor.matmul(
                        px[:],
                        ls[:, dc * 128:(dc + 1) * 128],
                        bdt_bg,
                        start=False,
                        stop=(n_acc == 2),
                    )
                    if has_bcov:
                        nc.tensor.matmul(
                            px[:],
                            bcov[:, dc * 128:(dc + 1) * 128],
                            bdt_bg,
                            start=False,
                            stop=True,
                        )

                    # diag term ds*noise: one broadcast multiply per
                    # [128,512] tile on the otherwise-idle GpSimd engine
                    tmp = tmp_pool.tile([128, 512], BF16)
                    n3 = nz[:, dc * 512:(dc + 1) * 512].rearrange(
                        "p (b s) -> p b s", b=8
                    )
                    ds3 = dst[dc][:, bg * 8:(bg + 1) * 8].rearrange(
                        "p (b o) -> p b o", o=1
                    )
                    n3b, ds3b = broadcast_tensor_aps(n3, ds3)
                    t3 = tmp[:].rearrange("p (b s) -> p b s", b=8)
                    nc.gpsimd.tensor_mul(t3, n3b, ds3b)

                    pt = pret_pool.tile([128, 512], BF16)
                    nc.vector.tensor_add(pt[:], px[:], tmp[:])
                    pret.append(pt)

                # D) logits for the 4 token-tiles of this b-group.
                # Two single-bank PSUM halves per ti; the PSUM->SBUF copies
                # split across ACT and DVE so neither gates the PE.
                for ti in range(4):
                    lo = lout_pool.tile([128, C], BF16)
                    pa_l = ps_l.tile([128, 512], F32)
                    pb_l = ps_l.tile([128, 512], F32)
                    pb = pb_l[:, 0:C - 512]
                    for dc in range(NDC):
                        stat = pret[dc][:, ti * 128:(ti + 1) * 128]
                        nc.tensor.matmul(
                            pa_l[:], stat, wcls_sb[:, dc * C:dc * C + 512],
                            start=(dc == 0), stop=(dc == NDC - 1),
                        )
                        nc.tensor.matmul(
                            pb[:],
                            stat,
                            wcls_sb[:, dc * C + 512:(dc + 1) * C],
                            start=(dc == 0), stop=(dc == NDC - 1),
                        )
                    if has_bcls:
                        nc.vector.tensor_add(lo[:, 0:512], pa_l[:], bcls[:, 0:512])
                        nc.vector.tensor_add(lo[:, 512:C], pb[:], bcls[:, 512:C])
                    else:
                        nc.scalar.activation(lo[:, 0:512], pa_l[:], AF.Copy)
                        nc.vector.tensor_copy(lo[:, 512:C], pb[:])
                    nc.sync.dma_start(
                        d_out[bg * 512 + ti * 128:bg * 512 + (ti + 1) * 128, :],
                        lo[:],
                    )

    nc.compile()
    return nc


def build_in_maps(
    features, W_cov, b_cov, W_diag, b_diag, W_cls, b_cls, noise_diag, noise_lr
):
    features = np.asarray(features, np.float32)
    W_cov = np.asarray(W_cov, np.float32)
    b_cov = np.asarray(b_cov, np.float32)
    W_diag = np.asarray(W_diag, np.float32)
    b_diag = np.asarray(b_diag, np.float32)
    W_cls = np.asarray(W_cls, np.float32)
    b_cls = np.asarray(b_cls, np.float32)
    noise_diag = np.asarray(noise_diag, np.float32)
    noise_lr = np.asarray(noise_lr, np.float32)

    has_bcov = bool(np.any(b_cov))
    has_bcls = bool(np.any(b_cls))

    # host-side layout prep (replicated weights)
    # W_cov[j=16d+r, e] -> [e, j'=512r+d]
    w_cov_t = np.ascontiguousarray(
        W_cov.reshape(D, R, D).transpose(2, 1, 0).reshape(D, D * R)
    ).astype(NP_BF16)
    w_diag_t = np.ascontiguousarray(W_diag.T).astype(NP_BF16)
    w_cls_t = np.ascontiguousarray(W_cls.T / 1.5).astype(NP_BF16)
    b_diag_col = np.ascontiguousarray(b_diag.reshape(NDC, 128).T)

    eye8 = np.kron(np.eye(8, dtype=np.float32), np.ones((1, 64), np.float32))
    eye8 = np.ascontiguousarray(eye8).astype(NP_BF16)

    common = {
        "w_cov_t": w_cov_t,
        "w_diag_t": w_diag_t,
        "w_cls_t": w_cls_t,
        "b_diag_col": b_diag_col,
        "eye8": eye8,
    }
    if has_bcov:
        # b_cov[j=16d+r] -> [r, d] tiled 8x along partitions -> [128, 512]
        bcov_r = b_cov.reshape(D, R).T
        common["bcov_tiled"] = np.ascontiguousarray(np.tile(bcov_r, (8, 1))).astype(
            NP_BF16
        )
    if has_bcls:
        common["bcls_bcast"] = np.ascontiguousarray(
            np.tile((b_cls / 1.5)[None, :], (128, 1))
        )

    in_maps = []
    for c in range(N_CORES):
        b0 = c * NB
        sl = slice(b0, b0 + NB)
        m = dict(common)
        m["f_gs"] = np.ascontiguousarray(
            features[sl].reshape(NBG, 8, D).transpose(1, 0, 2).reshape(8, NBG * D)
        ).astype(NP_BF16)
        m["f_t"] = np.ascontiguousarray(features[sl].T).astype(NP_BF16)
        # block-diag noise_lr: per bg a [128=(g,r), 512=(g,s)] tile
        nlr_c = noise_lr[sl].reshape(NBG, 8, S, R)
        bdt = np.zeros((NBG, 8, R, 8, S), np.float32)
        for g in range(8):
            bdt[:, g, :, g, :] = nlr_c[:, g].transpose(0, 2, 1)
        m["bdt"] = np.ascontiguousarray(
            bdt.reshape(NBG, 128, 512).transpose(1, 0, 2).reshape(128, NBG * 512)
        ).astype(NP_BF16)
        # noise transposed to [d, t]
        m["noise_t"] = np.ascontiguousarray(
            noise_diag[sl].reshape(NT, D).T
        ).astype(NP_BF16)
        in_maps.append(m)

    return in_maps, has_bcov, has_bcls


def kernel(**inputs):
    in_maps, has_bcov, has_bcls = build_in_maps(**inputs)
    nc = _build(has_bcov, has_bcls)
    res = run_bass_kernel_spmd(nc, in_maps, list(range(N_CORES)))
    out = np.concatenate(
        [np.asarray(res.results[c]["out"]).astype(np.float32) for c in range(N_CORES)],
        axis=0,
    )
    return out.reshape(B_FULL, S, C)


if __name__ == "__main__":
    rng = np.random.default_rng(0)
    ins = {
        "features": rng.standard_normal((B_FULL, D), np.float32),
        "W_cov": rng.standard_normal((D * R, D), np.float32) / np.sqrt(D),
        "b_cov": np.zeros((D * R,), np.float32),
        "W_diag": rng.standard_normal((D, D), np.float32) / np.sqrt(D),
        "b_diag": np.zeros((D,), np.float32),
        "W_cls": rng.standard_normal((C, D), np.float32) / np.sqrt(D),
        "b_cls": np.zeros((C,), np.float32),
        "noise_diag": rng.standard_normal((B_FULL, S, D), np.float32),
        "noise_lr": rng.standard_normal((B_FULL, S, R), np.float32),
    }
    out = kernel(**ins)
    print(out.shape, out.dtype)


# revision 32
# speedup vs baseline: 1.1963x; 1.0550x over previous
"""HETXLHead forward on 8 Trainium2 NeuronCores (Bass/Tile).

Reference computation (per batch row b, S=64 samples, D=512, R=16, C=1000):
    lrc   = (f @ W_cov.T + b_cov).reshape(B, D, R)
    ds    = softplus(f @ W_diag.T + b_diag) + 1e-3
    smp   = einsum('bdr,bsr->bsd', lrc, nlr) + ds[:,None,:] * nd
    out   = ((f[:,None,:] + smp) @ W_cls.T + b_cls) / 1.5

Sharding: data-parallel over B=1024 -> 128 rows per core; weights replicated.

Device pipeline per core (tokens t = 64*b + s, 8192 per core; all heavy
streams host-cast to bf16 to halve HBM traffic):
  A) cov[b, j'] = f @ W_cov'.T via PE, j' = 512*r + d (r-major, host-permuted)
  B) dsT[d, b] = softplus(W_diag @ f.T + b_diag) + 1e-3 (ACT Softplus direct)
  C) one SBUF->SBUF shuffle DMA per b-group of 8 turns cov rows into the
     block-diag lhsT ls[(g,r), d]; PSUM X = ls.T @ bdt (host-built block-diag
     noise_lr) + f via a K=8 ones-block matmul; noise_diag arrives HOST-
     TRANSPOSED [d, t] so the diag term is one broadcast DVE multiply per
     [128,512] tile (dsT column broadcast over each b's 64 sample columns);
     DVE adds PSUM X + diag -> pret (bf16).
  D) logits[t, c] = pret.T @ (W_cls.T/1.5) in bf16, ACT-copy PSUM->SBUF f32,
     one batched DMA out per b-group.
"""

import os
import sys

for _p in ("/opt/trn_rl_repo", "/opt/pypackages"):
    if os.path.isdir(_p) and _p not in sys.path:
        sys.path.insert(0, _p)

import numpy as np

import concourse.bass as bass
import concourse.bacc as bacc
import concourse.mybir as mybir
import concourse.tile as tile
from concourse.bass import broadcast_tensor_aps
from concourse.bass_utils import run_bass_kernel_spmd

F32 = mybir.dt.float32
BF16 = mybir.dt.bfloat16
NP_BF16 = mybir.dt.np(BF16)
AF = mybir.ActivationFunctionType

N_CORES = 8
B_FULL, D, R, S, C = 1024, 512, 16, 64, 1000
NB = B_FULL // N_CORES        # 128 batch rows per core
NT = NB * S                   # 8192 tokens per core
NDC = D // 128                # 4 d-chunks
NEC = D // 128                # 4 e-chunks (contraction of the input feature dim)
NBG = NB // 8                 # 16 b-groups of 8
NJC = R                       # 16 j'-chunks of 512 (one per r)
MIN_SCALE = 1e-3


def _build(has_bcov: bool, has_bcls: bool):
    nc = bacc.Bacc(None, target_bir_lowering=False, debug=True)

    d_ft = nc.dram_tensor("f_t", [D, NB], BF16, kind="ExternalInput")
    d_fgs = nc.dram_tensor("f_gs", [8, NBG * D], BF16, kind="ExternalInput")
    d_eye8 = nc.dram_tensor("eye8", [8, 512], BF16, kind="ExternalInput")
    d_wcov = nc.dram_tensor("w_cov_t", [D, D * R], BF16, kind="ExternalInput")
    d_wdiag = nc.dram_tensor("w_diag_t", [D, D], BF16, kind="ExternalInput")
    d_wcls = nc.dram_tensor("w_cls_t", [D, C], BF16, kind="ExternalInput")
    d_bdiag = nc.dram_tensor("b_diag_col", [128, NDC], F32, kind="ExternalInput")
    d_bdt = nc.dram_tensor("bdt", [128, NBG * 512], BF16, kind="ExternalInput")
    d_noise = nc.dram_tensor("noise_t", [D, NT], BF16, kind="ExternalInput")
    if has_bcov:
        d_bcov = nc.dram_tensor("bcov_tiled", [128, D], BF16, kind="ExternalInput")
    if has_bcls:
        d_bcls = nc.dram_tensor("bcls_bcast", [128, C], F32, kind="ExternalInput")
    d_out = nc.dram_tensor("out", [NT, C], BF16, kind="ExternalOutput")

    with tile.TileContext(nc) as tc:
        with (
            tc.tile_pool(name="const", bufs=1) as const_pool,
            tc.tile_pool(name="wstream", bufs=4) as wstream_pool,
            tc.tile_pool(name="ls", bufs=6) as ls_pool,
            tc.tile_pool(name="noise", bufs=4) as noise_pool,
            tc.tile_pool(name="tmp", bufs=6) as tmp_pool,
            tc.tile_pool(name="pret", bufs=8) as pret_pool,
            tc.tile_pool(name="lout", bufs=6) as lout_pool,
            tc.tile_pool(name="ps_x", bufs=4, space="PSUM") as ps_x,
            tc.tile_pool(name="ps_l", bufs=2, space="PSUM") as ps_l,
        ):
            # ---- constants into SBUF (batched loads) ----
            # W_cov-dependent work gates the whole pipeline, so its stream
            # (scalar ring) starts first; bulky consts not needed until the
            # bg loop (wcls, bdt) are emitted after the cov loop below.
            ft_sb = const_pool.tile([128, NEC * NB], BF16, tag="ft")
            nc.sync.dma_start(
                ft_sb[:], d_ft.rearrange("(ec p) b -> p ec b", ec=NEC)
            )
            wd_sb = const_pool.tile([128, NEC * D], BF16, tag="wd")
            nc.sync.dma_start(
                wd_sb[:], d_wdiag.rearrange("(ec p) d -> p ec d", ec=NEC)
            )
            bdiag = const_pool.tile([128, NDC], F32, tag="bdiag")
            nc.sync.dma_start(bdiag[:], d_bdiag[:, :])
            fgs = const_pool.tile([8, NBG * D], BF16, tag="fgs")
            nc.sync.dma_start(fgs[:], d_fgs[:, :])
            eye8 = const_pool.tile([8, 512], BF16, tag="eye8")
            nc.sync.dma_start(eye8[:], d_eye8[:, :])

            def ft(ec):
                return ft_sb[:, ec * NB:(ec + 1) * NB]

            # ---- B) dsT[d, b] = softplus(W_diag @ f.T + b_diag) + 1e-3 ----
            dst = []
            for dc in range(NDC):
                pd_full = ps_x.tile([128, 512], F32, tag="px")
                pd = pd_full[:, 0:NB]
                for ec in range(NEC):
                    nc.tensor.matmul(
                        pd[:],
                        wd_sb[:, ec * D + dc * 128:ec * D + (dc + 1) * 128],
                        ft(ec),
                        start=(ec == 0),
                        stop=(ec == NEC - 1),
                    )
                t = const_pool.tile([128, NB], F32, tag=f"dst{dc}")
                # softplus(x) = Ln(1 + Exp(x)); x = pd + b_diag column
                nc.scalar.activation(
                    t[:], pd[:], AF.Exp, bias=bdiag[:, dc:dc + 1], scale=1.0
                )
                nc.vector.tensor_scalar_add(t[:], t[:], 1.0)
                nc.scalar.activation(t[:], t[:], AF.Ln)
                nc.vector.tensor_scalar_add(t[:], t[:], MIN_SCALE)
                dst.append(t)

            # ---- A) cov[b, j'] (j' = 512r + d, r-major) ----
            cov = const_pool.tile([NB, D * R], BF16, tag="cov")
            wcov_r = d_wcov.rearrange("(ec p) j -> p ec j", ec=NEC)
            for jc in range(NJC):
                wt = wstream_pool.tile([128, NEC * 512], BF16)
                nc.scalar.dma_start(
                    wt[:], wcov_r[:, :, jc * 512:(jc + 1) * 512]
                )
                pa = ps_x.tile([128, 512], F32, tag="px")
                for ec in range(NEC):
                    nc.tensor.matmul(
                        pa[:], ft(ec), wt[:, ec * 512:(ec + 1) * 512],
                        start=(ec == 0), stop=(ec == NEC - 1),
                    )
                nc.vector.tensor_copy(cov[:, jc * 512:(jc + 1) * 512], pa[:])

            # bulky bg-loop consts after the cov stream so they don't steal
            # its HBM bandwidth (needed by ~t=28us; ready well before)
            bdt_sb = const_pool.tile([128, NBG * 512], BF16, tag="bdt")
            nc.sync.dma_start(bdt_sb[:], d_bdt[:, :])
            wcls_sb = const_pool.tile([128, NDC * C], BF16, tag="wcls")
            nc.sync.dma_start(
                wcls_sb[:], d_wcls.rearrange("(dc p) c -> p dc c", dc=NDC)
            )
            if has_bcov:
                bcov = const_pool.tile([128, D], BF16, tag="bcov")
                nc.sync.dma_start(bcov[:], d_bcov[:, :])
            if has_bcls:
                bcls = const_pool.tile([128, C], F32, tag="bcls")
                nc.sync.dma_start(bcls[:], d_bcls[:, :])

            # ---- C + D pipeline over b-groups ----
            noise_r = d_noise.rearrange("(dc p) t -> p dc t", dc=NDC)
            for bg in range(NBG):
                # block-diag lhsT ls[(g,r), d] via one SBUF->SBUF shuffle DMA
                ls = ls_pool.tile([128, D], BF16)
                nc.gpsimd.dma_start(
                    ls[:],
                    cov[bg * 8:(bg + 1) * 8, :].rearrange("p (r d) -> p r d", r=R),
                )
                # noise tiles for this b-group's 512 tokens, [d, t] layout
                nz = noise_pool.tile([128, NDC * 512], BF16)
                nc.sync.dma_start(
                    nz[:], noise_r[:, :, bg * 512:(bg + 1) * 512]
                )

                bdt_bg = bdt_sb[:, bg * 512:(bg + 1) * 512]
                pret = []
                for dc in range(NDC):
                    px = ps_x.tile([128, 512], F32, tag="px")
                    n_acc = 3 if has_bcov else 2
                    nc.tensor.matmul(
                        px[:],
                        fgs[:, bg * D + dc * 128:bg * D + (dc + 1) * 128],
                        eye8[:],
                        start=True,
                        stop=(n_acc == 1),
                    )
                    nc.tensor.matmul(
                        px[:],
                        ls[:, dc * 128:(dc + 1) * 128],
                        bdt_bg,
                        start=False,
                        stop=(n_acc == 2),
                    )
                    if has_bcov:
                        nc.tensor.matmul(
                            px[:],
                            bcov[:, dc * 128:(dc + 1) * 128],
                            bdt_bg,
                            start=False,
                            stop=True,
                        )

                    # diag term ds*noise: one broadcast multiply per
                    # [128,512] tile on the otherwise-idle GpSimd engine
                    tmp = tmp_pool.tile([128, 512], BF16)
                    n3 = nz[:, dc * 512:(dc + 1) * 512].rearrange(
                        "p (b s) -> p b s", b=8
                    )
                    ds3 = dst[dc][:, bg * 8:(bg + 1) * 8].rearrange(
                        "p (b o) -> p b o", o=1
                    )
                    n3b, ds3b = broadcast_tensor_aps(n3, ds3)
                    t3 = tmp[:].rearrange("p (b s) -> p b s", b=8)
                    nc.gpsimd.tensor_mul(t3, n3b, ds3b)

                    pt = pret_pool.tile([128, 512], BF16)
                    nc.vector.tensor_add(pt[:], px[:], tmp[:])
                    pret.append(pt)

                # D) logits for the 4 token-tiles of this b-group.
                # Two single-bank PSUM halves per ti; the PSUM->SBUF copies
                # split across ACT and DVE so neither gates the PE.
                for ti in range(4):
                    lo = lout_pool.tile([128, C], BF16)
                    pa_l = ps_l.tile([128, 512], F32)
                    pb_l = ps_l.tile([128, 512], F32)
                    pb = pb_l[:, 0:C - 512]
                    for dc in range(NDC):
                        stat = pret[dc][:, ti * 128:(ti + 1) * 128]
                        nc.tensor.matmul(
                            pa_l[:], stat, wcls_sb[:, dc * C:dc * C + 512],
                            start=(dc == 0), stop=(dc == NDC - 1),
                        )
                        nc.tensor.matmul(
                            pb[:],
                            stat,
                            wcls_sb[:, dc * C + 512:(dc + 1) * C],
                            start=(dc == 0), stop=(dc == NDC - 1),
                        )
                    if has_bcls:
                        nc.vector.tensor_add(lo[:, 0:512], pa_l[:], bcls[:, 0:512])
                        nc.vector.tensor_add(lo[:, 512:C], pb[:], bcls[:, 512:C])
                    else:
                        nc.scalar.activation(lo[:, 0:512], pa_l[:], AF.Copy)
                        nc.vector.tensor_copy(lo[:, 512:C], pb[:])
                    # scalar ring: idle after the wcov stream, and its
                    # copy-sem waits can't block the noise loads (sync ring)
                    nc.scalar.dma_start(
                        d_out[bg * 512 + ti * 128:bg * 512 + (ti + 1) * 128, :],
                        lo[:],
                    )

    nc.compile()
    return nc


def build_in_maps(
    features, W_cov, b_cov, W_diag, b_diag, W_cls, b_cls, noise_diag, noise_lr
):
    features = np.asarray(features, np.float32)
    W_cov = np.asarray(W_cov, np.float32)
    b_cov = np.asarray(b_cov, np.float32)
    W_diag = np.asarray(W_diag, np.float32)
    b_diag = np.asarray(b_diag, np.float32)
    W_cls = np.asarray(W_cls, np.float32)
    b_cls = np.asarray(b_cls, np.float32)
    noise_diag = np.asarray(noise_diag, np.float32)
    noise_lr = np.asarray(noise_lr, np.float32)

    has_bcov = bool(np.any(b_cov))
    has_bcls = bool(np.any(b_cls))

    # host-side layout prep (replicated weights)
    # W_cov[j=16d+r, e] -> [e, j'=512r+d]
    w_cov_t = np.ascontiguousarray(
        W_cov.reshape(D, R, D).transpose(2, 1, 0).reshape(D, D * R)
    ).astype(NP_BF16)
    w_diag_t = np.ascontiguousarray(W_diag.T).astype(NP_BF16)
    w_cls_t = np.ascontiguousarray(W_cls.T / 1.5).astype(NP_BF16)
    b_diag_col = np.ascontiguousarray(b_diag.reshape(NDC, 128).T)

    eye8 = np.kron(np.eye(8, dtype=np.float32), np.ones((1, 64), np.float32))
    eye8 = np.ascontiguousarray(eye8).astype(NP_BF16)

    common = {
        "w_cov_t": w_cov_t,
        "w_diag_t": w_diag_t,
        "w_cls_t": w_cls_t,
        "b_diag_col": b_diag_col,
        "eye8": eye8,
    }
    if has_bcov:
        # b_cov[j=16d+r] -> [r, d] tiled 8x along partitions -> [128, 512]
        bcov_r = b_cov.reshape(D, R).T
        common["bcov_tiled"] = np.ascontiguousarray(np.tile(bcov_r, (8, 1))).astype(
            NP_BF16
        )
    if has_bcls:
        common["bcls_bcast"] = np.ascontiguousarray(
            np.tile((b_cls / 1.5)[None, :], (128, 1))
        )

    in_maps = []
    for c in range(N_CORES):
        b0 = c * NB
        sl = slice(b0, b0 + NB)
        m = dict(common)
        m["f_gs"] = np.ascontiguousarray(
            features[sl].reshape(NBG, 8, D).transpose(1, 0, 2).reshape(8, NBG * D)
        ).astype(NP_BF16)
        m["f_t"] = np.ascontiguousarray(features[sl].T).astype(NP_BF16)
        # block-diag noise_lr: per bg a [128=(g,r), 512=(g,s)] tile
        nlr_c = noise_lr[sl].reshape(NBG, 8, S, R)
        bdt = np.zeros((NBG, 8, R, 8, S), np.float32)
        for g in range(8):
            bdt[:, g, :, g, :] = nlr_c[:, g].transpose(0, 2, 1)
        m["bdt"] = np.ascontiguousarray(
            bdt.reshape(NBG, 128, 512).transpose(1, 0, 2).reshape(128, NBG * 512)
        ).astype(NP_BF16)
        # noise transposed to [d, t]
        m["noise_t"] = np.ascontiguousarray(
            noise_diag[sl].reshape(NT, D).T
        ).astype(NP_BF16)
        in_maps.append(m)

    return in_maps, has_bcov, has_bcls


def kernel(**inputs):
    in_maps, has_bcov, has_bcls = build_in_maps(**inputs)
    nc = _build(has_bcov, has_bcls)
    res = run_bass_kernel_spmd(nc, in_maps, list(range(N_CORES)))
    out = np.concatenate(
        [np.asarray(res.results[c]["out"]).astype(np.float32) for c in range(N_CORES)],
        axis=0,
    )
    return out.reshape(B_FULL, S, C)


if __name__ == "__main__":
    rng = np.random.default_rng(0)
    ins = {
        "features": rng.standard_normal((B_FULL, D), np.float32),
        "W_cov": rng.standard_normal((D * R, D), np.float32) / np.sqrt(D),
        "b_cov": np.zeros((D * R,), np.float32),
        "W_diag": rng.standard_normal((D, D), np.float32) / np.sqrt(D),
        "b_diag": np.zeros((D,), np.float32),
        "W_cls": rng.standard_normal((C, D), np.float32) / np.sqrt(D),
        "b_cls": np.zeros((C,), np.float32),
        "noise_diag": rng.standard_normal((B_FULL, S, D), np.float32),
        "noise_lr": rng.standard_normal((B_FULL, S, R), np.float32),
    }
    out = kernel(**ins)
    print(out.shape, out.dtype)
